# revision 9
# baseline (speedup 1.0000x reference)
"""ConditionAwareAdaIN Trainium2 kernel (bf16 rewrite).

Reference computation (B=16, C=256, L=1024, U=64, Q=64):
    nx    = InstanceNorm1d(x)                       # per-(b,c) stats over L
    A     = einsum('bu,cuq->bcq', u_i, W.reshape(2C,U,Q))
    style = einsum('bcq,bql->bcl', A, e_qid)
    gamma, beta = split(style + V@t + bias, 2, axis=1)
    out   = (1 + gamma) * nx + beta

Sharding: 2-way over batch x 4-way over channels -> 8 cores, each owning
8 samples x 64 channels (its slice of gamma AND beta rows of W/V/bias).

All bulk tensors move as bf16 (DMA time on the cost model is serialized
bytes/360GB/s, so halving bytes halves the DMA floor); PSUM accumulation
stays fp32, output is bf16 and upcast on host.

Per-core device kernel:
  stage 1: A in c2-major layout via 64 tiny matmuls (one per q):
           pa[c2, (s, sp, q)] += wt_q[u, c2]^T @ ui[u, b].  PE transposes
           (one per sample pair) then produce the block-diagonal stage-2
           lhsT with q on partitions; no DRAM bounce, tiny evacuations.
  stage 2: per sample-pair, block-diagonal style matmuls (K=128) + a K=3
           accumulating matmul folding V*t, bias and the "+1" of
           (1+gamma).
  norm:    bn_stats/bn_aggr per pair tile (2 samples x 64 ch = 128 rows);
           xm = (x - mean) * rstd on DVE (4x bf16 mode), params
           evacuated PSUM->bf16 on ACT, then out = xm*(1+gamma) + beta
           as two 2x-mode DVE tensor_tensor ops.
"""

import json

import numpy as np
import ml_dtypes

for _p in ("/opt/trn_rl_repo", "/root/.axon_site/_ro/trn_rl_repo"):
    import sys as _sys

    if _p not in _sys.path:
        _sys.path.append(_p)

import concourse.bass as bass
import concourse.mybir as mybir
from concourse.tile import TileContext
from concourse.bass_utils import run_bass_kernel_spmd
from concourse import masks


def _split_sync_waits(raw: bytes, keep: int = 1) -> bytes:
    """Walrus in this env accepts at most one sync wait per TPB instruction.

    Tile packs several waits into sync_info.on_wait; re-emit the excess as
    standalone single-wait EventSemaphore instructions (what wait_ge emits)
    immediately before the instruction, in the same engine stream.
    """
    bir = json.loads(raw)
    n = 0
    for fn in bir["functions"]:
        for blk in fn["blocks"]:
            out = []
            for ins in blk["instructions"]:
                si = ins.get("sync_info")
                ws = si.get("on_wait") if si else None
                if ws and len(ws) > keep:
                    for w in ws[: len(ws) - keep]:
                        n += 1
                        out.append(
                            {
                                "debug": ins.get("debug", 0),
                                "engine": ins["engine"],
                                "ins": [],
                                "outs": [],
                                "name": f"evw-{n}",
                                "opcode": "EventSemaphore",
                                "sync_info": {"on_update": [], "on_wait": [w]},
                            }
                        )
                    si["on_wait"] = ws[len(ws) - keep :]
                out.append(ins)
            blk["instructions"] = out
    return json.dumps(bir).encode()


class _Bass(bass.Bass):
    def to_json_bytes(self) -> bytes:
        return _split_sync_waits(super().to_json_bytes())


B, C, L = 16, 256, 1024
U, Q = 64, 64
EPS = 1e-5
N_CORES = 8
BG, CG = 2, 4          # batch groups x channel groups
BPC = B // BG          # samples per core = 8
CPC = C // CG          # channels per core = 64
NPAIR = BPC // 2       # sample pairs per core = 4

FP32 = mybir.dt.float32
BF16 = mybir.dt.bfloat16

_CACHE = {}


def _build_nc(detect_races: bool = True):
    nc = _Bass(detect_race_conditions=detect_races)

    # x / e pair-group tiles: [g, 128 rows (2 samples x 64 ch/q), 2 pairs x L]
    xg_in = nc.dram_tensor("xg_s", [2, 128, 2 * L], BF16, kind="ExternalInput")
    eg_in = nc.dram_tensor("eg_s", [2, 128, 2 * L], BF16, kind="ExternalInput")
    # wt: stage-1 lhsT per q: wt[u, q*128 + c2] = W2[c2, u, q]
    wt_in = nc.dram_tensor("wt_s", [64, 64 * 128], BF16, kind="ExternalInput")
    ui_in = nc.dram_tensor("ui_s", [64, BPC], BF16, kind="ExternalInput")
    # sm: [r2 (4 pairs x 1024) | l2 (256)] on 3 partitions (fp32)
    sm_in = nc.dram_tensor("sm2", [3, NPAIR * L + 256], FP32, kind="ExternalInput")
    out_d = nc.dram_tensor("out_s", [2, 128, 2 * L], BF16, kind="ExternalOutput")

    AF = mybir.ActivationFunctionType
    OP = mybir.AluOpType
    F32R = mybir.dt.float32r

    with TileContext(nc) as tc:
        with (
            tc.tile_pool(name="persist", bufs=1) as persist,
            tc.tile_pool(name="stat", bufs=8) as stat,
            tc.tile_pool(name="work", bufs=4) as work,
            tc.tile_pool(name="psM", bufs=4, space="PSUM") as psM,
        ):
            # small local tiles first (no DMA deps) so PE warm-up can start
            dum = persist.tile([64, 512], BF16, tag="dum")
            nc.vector.memset(dum, 0.0)
            eps_t = persist.tile([128, 1], FP32, tag="eps")
            nc.vector.memset(eps_t, EPS)
            idn = persist.tile([128, 128], BF16, tag="idn")
            masks.make_identity(nc, idn[:, :])
            # block-diagonal stage-2 lhsT, zero blocks stay zero
            lt = persist.tile([128, NPAIR, 256], BF16, tag="lt")
            nc.gpsimd.memset(lt[:, :, :], 0.0)

            # ---- input DMAs (DMA device is serialized; order matters):
            # ui (tiny) -> x group 0 (unblocks stats) -> wt (unblocks the
            # stage-1 -> lt chain) -> e group 0 + sm (unblocks stage 2) ->
            # x/e group 1.
            sm = persist.tile([3, NPAIR * L + 256], F32R, tag="sm")
            nc.sync.dma_start(out=sm, in_=sm_in[:, :].bitcast(F32R))
            r2 = sm[:, 0 : NPAIR * L].rearrange("k (s l) -> k s l", s=NPAIR)
            l2 = sm[:, NPAIR * L : NPAIR * L + 256]
            uit = persist.tile([64, BPC], BF16, tag="uit")
            nc.sync.dma_start(out=uit, in_=ui_in[:, :])
            xgt = persist.tile([128, 2, 2 * L], BF16, tag="xgt")
            egt = persist.tile([128, 2, 2 * L], BF16, tag="egt")
            nc.sync.dma_start(out=xgt[:, 0, :], in_=xg_in[0, :, :])
            wt = persist.tile([64, 64 * 128], BF16, tag="wt")
            nc.sync.dma_start(out=wt[:, 0 : 32 * 128], in_=wt_in[:, 0 : 32 * 128])
            nc.sync.dma_start(out=wt[:, 32 * 128 :], in_=wt_in[:, 32 * 128 :])
            nc.sync.dma_start(out=egt[:, 0, :], in_=eg_in[0, :, :])
            nc.sync.dma_start(out=egt[:, 1, :], in_=eg_in[1, :, :])
            nc.sync.dma_start(out=xgt[:, 1, 0:L], in_=xg_in[1, :, 0:L])
            nc.sync.dma_start(out=xgt[:, 1, L:], in_=xg_in[1, :, L:])

            ott = persist.tile([128, 2, 2 * L], BF16, tag="ott")

            # ---- PE warm-up: start the p-state ramp clock early ----
            # pa cells are all rewritten with start=True by stage 1.
            pa = psM.tile([128, 4, 2, 64], FP32, tag="ps", name="pa")
            paw = pa.rearrange("p s h q -> p (s h q)")
            for wu in range(3):
                nc.tensor.matmul(
                    paw[0:8, 0:512],
                    lhsT=dum[:, 0:8],
                    rhs=dum[:, 0:512],
                    start=True,
                    stop=True,
                )

            # ---- stage 1: pa[c2, (s, sp, q)] = sum_u wt_q[u, c2] ui[u, b] ----
            # b = 2s + sp; out free dims (s:4, sp:2) are strided, offset q.
            a_sb = persist.tile([128, 512], BF16, tag="a_sb")
            asv = a_sb.rearrange("p (s h q) -> p s h q", s=NPAIR, h=2)
            for qh in range(2):
                for q in range(qh * 32, qh * 32 + 32):
                    nc.tensor.matmul(
                        pa[:, :, :, q],
                        lhsT=wt[:, q * 128 : (q + 1) * 128],
                        rhs=uit.rearrange("u (s h) -> u s h", s=NPAIR),
                        start=True,
                        stop=True,
                    )
                # evacuate A half to SBUF bf16, layout [c2, (s, sp*64+q)]
                nc.scalar.activation(
                    out=asv[:, :, :, qh * 32 : qh * 32 + 32],
                    in_=pa[:, :, :, qh * 32 : qh * 32 + 32],
                    func=AF.Copy,
                )

            # per pair: PE transpose [c2, (sp,q)] -> [(sp,q), c2] (bf16 PSUM)
            # same tag as pa: reuses its bank once evac'd
            pt = psM.tile([128, 4, 128], BF16, tag="ps", name="pt")
            for s in range(NPAIR):
                nc.tensor.transpose(
                    pt[:, s, :], a_sb[:, s * 128 : (s + 1) * 128], idn[:, :]
                )
            # scatter into the block-diagonal lhsT:
            #   lt[sp*64+q, s, gb*128 + sp*64 + c'] = A[2s+sp, gb*64+c', q]
            lt4 = lt.rearrange("p s (gb c) -> p s gb c", gb=2)
            pt4 = pt.rearrange("p s (gb c) -> p s gb c", gb=2)
            for s in range(NPAIR):
                for sp in range(2):
                    rows = slice(sp * 64, sp * 64 + 64)
                    nc.vector.tensor_copy(
                        out=lt4[rows, s, :, sp * 64 : sp * 64 + 64],
                        in_=pt4[rows, s, :, :],
                    )

            # ---- stage 2 + norm, per sample pair ----
            od = out_d.rearrange("g p (i l) -> g p i l", i=2)
            for s in range(NPAIR):
                g, i = divmod(s, 2)
                xt = xgt[:, g, i * L : (i + 1) * L]
                et = egt[:, g, i * L : (i + 1) * L]

                st = stat.tile([128, 2, 6], FP32, tag="st")
                nc.vector.bn_stats(st[:, 0, :], xt[:, 0:512])
                nc.vector.bn_stats(st[:, 1, :], xt[:, 512:1024])
                mv = stat.tile([128, 2], FP32, tag="mv")
                nc.vector.bn_aggr(mv, st)
                rstd = stat.tile([128, 1], FP32, tag="rstd")
                nc.scalar.activation(
                    out=rstd, in_=mv[:, 1:2], func=AF.Sqrt, bias=eps_t, scale=1.0
                )
                nc.vector.reciprocal(rstd, rstd)

                # xm = (x - mean) * rstd   (DVE 4x bf16 mode)
                xm = work.tile([128, L], BF16, tag="xm")
                nc.vector.tensor_scalar(
                    out=xm,
                    in0=xt,
                    scalar1=mv[:, 0:1],
                    scalar2=rstd,
                    op0=OP.subtract,
                    op1=OP.mult,
                )

                pm = psM.tile([128, L], FP32, tag="ps", name=f"pm{s}")
                pb = psM.tile([128, L], FP32, tag="ps", name=f"pb{s}")
                for h in range(2):
                    cols = slice(h * 512, (h + 1) * 512)
                    nc.tensor.matmul(
                        pm[:, cols],
                        lhsT=lt[:, s, 0:128],
                        rhs=et[:, cols],
                        start=True,
                        stop=False,
                    )
                    nc.tensor.matmul(
                        pm[:, cols],
                        lhsT=l2[:, 0:128],
                        rhs=r2[:, s, cols],
                        start=False,
                        stop=True,
                    )
                    nc.tensor.matmul(
                        pb[:, cols],
                        lhsT=lt[:, s, 128:256],
                        rhs=et[:, cols],
                        start=True,
                        stop=False,
                    )
                    nc.tensor.matmul(
                        pb[:, cols],
                        lhsT=l2[:, 128:256],
                        rhs=r2[:, s, cols],
                        start=False,
                        stop=True,
                    )

                otv = ott[:, g, i * L : (i + 1) * L]
                if s < NPAIR - 1:
                    # evacuate params PSUM -> bf16 SBUF (ACT), then combine
                    # on DVE with 2x bf16 tensor_tensor ops:
                    #   ot = xm * (1+gamma) + beta
                    mg = work.tile([128, L], BF16, tag="mg")
                    nc.scalar.activation(out=mg, in_=pm, func=AF.Copy)
                    mb = work.tile([128, L], BF16, tag="mb")
                    nc.scalar.activation(out=mb, in_=pb, func=AF.Copy)
                    nc.vector.tensor_tensor(out=otv, in0=xm, in1=mg, op=OP.mult)
                    nc.vector.tensor_tensor(out=otv, in0=otv, in1=mb, op=OP.add)
                else:
                    # last pair: DVE-direct from PSUM, skipping the ACT
                    # evac queue (DVE is idle by now; latency beats modes)
                    prd = work.tile([128, L], BF16, tag="mg")
                    nc.vector.scalar_tensor_tensor(
                        out=prd, in0=pm, scalar=1.0, in1=xm,
                        op0=OP.mult, op1=OP.mult,
                    )
                    nc.vector.tensor_tensor(out=otv, in0=prd, in1=pb, op=OP.add)

                nc.sync.dma_start(out=od[g, :, i, :], in_=otv)

    return nc


def _prep_core_inputs(core, x, u_i, e_qid, t, W, V, bias):
    bg, cg = divmod(core, CG)
    bs = slice(bg * BPC, (bg + 1) * BPC)
    rg = slice(cg * CPC, (cg + 1) * CPC)
    rb = slice(C + cg * CPC, C + (cg + 1) * CPC)
    bf = ml_dtypes.bfloat16

    # x / e pair tiles -> groups of 2 pairs side by side
    xp = x[bs, rg, :].reshape(NPAIR, 128, L)
    ep = e_qid[bs].reshape(NPAIR, 128, L)
    xg = np.concatenate([xp[0::2], xp[1::2]], axis=2)   # [2, 128, 2L]
    eg = np.concatenate([ep[0::2], ep[1::2]], axis=2)

    w2 = np.concatenate([W[rg], W[rb]], axis=0)          # (128, 4096) c2=[g|b]
    wr = w2.reshape(128, U, Q)                           # [c2, u, q]
    wt = np.ascontiguousarray(wr.transpose(1, 2, 0)).reshape(64, Q * 128)

    ui_s = np.ascontiguousarray(u_i[bs].T)               # (64, 8)

    vg, vb = V[rg, 0], V[rb, 0]
    bgm, bbt = bias[rg], bias[rb]
    l2 = np.zeros((3, 256), np.float32)
    l2[0, 0:64] = vg
    l2[1, 64:128] = vg
    l2[2, 0:64] = 1.0 + bgm
    l2[2, 64:128] = 1.0 + bgm
    l2[0, 128:192] = vb
    l2[1, 192:256] = vb
    l2[2, 128:192] = bbt
    l2[2, 192:256] = bbt

    r2 = np.empty((3, NPAIR, L), np.float32)
    for s in range(NPAIR):
        r2[0, s] = t[bg * BPC + 2 * s, 0]
        r2[1, s] = t[bg * BPC + 2 * s + 1, 0]
    r2[2] = 1.0
    sm = np.concatenate([r2.reshape(3, NPAIR * L), l2], axis=1)

    return {
        "xg_s": np.ascontiguousarray(xg).astype(bf),
        "eg_s": np.ascontiguousarray(eg).astype(bf),
        "wt_s": wt.astype(bf),
        "ui_s": ui_s.astype(bf),
        "sm2": np.ascontiguousarray(sm, dtype=np.float32),
    }


def kernel(x, u_i, e_qid, t, W, V, bias):
    x = np.asarray(x, np.float32)
    u_i = np.asarray(u_i, np.float32)
    e_qid = np.asarray(e_qid, np.float32)
    t = np.asarray(t, np.float32)
    W = np.asarray(W, np.float32)
    V = np.asarray(V, np.float32)
    bias = np.asarray(bias, np.float32)

    if "nc" not in _CACHE:
        _CACHE["nc"] = _build_nc()
    nc = _CACHE["nc"]

    in_maps = [
        _prep_core_inputs(i, x, u_i, e_qid, t, W, V, bias) for i in range(N_CORES)
    ]
    results = run_bass_kernel_spmd(nc, in_maps, list(range(N_CORES))).results

    out = np.empty((B, C, L), np.float32)
    for i in range(N_CORES):
        bg, cg = divmod(i, CG)
        res = np.asarray(results[i]["out_s"], dtype=np.float32)  # [2, 128, 2L]
        # [g, (sp c), (i l)] -> sample b = 4g + 2i + sp
        res = res.reshape(2, 2, CPC, 2, L).transpose(0, 3, 1, 2, 4)
        out[bg * BPC : (bg + 1) * BPC, cg * CPC : (cg + 1) * CPC, :] = res.reshape(
            BPC, CPC, L
        )
    return out


# revision 10
# speedup vs baseline: 1.0107x; 1.0107x over previous
"""ConditionAwareAdaIN Trainium2 kernel (bf16 rewrite).

Reference computation (B=16, C=256, L=1024, U=64, Q=64):
    nx    = InstanceNorm1d(x)                       # per-(b,c) stats over L
    A     = einsum('bu,cuq->bcq', u_i, W.reshape(2C,U,Q))
    style = einsum('bcq,bql->bcl', A, e_qid)
    gamma, beta = split(style + V@t + bias, 2, axis=1)
    out   = (1 + gamma) * nx + beta

Sharding: 2-way over batch x 4-way over channels -> 8 cores, each owning
8 samples x 64 channels (its slice of gamma AND beta rows of W/V/bias).

All bulk tensors move as bf16 (DMA time on the cost model is serialized
bytes/360GB/s, so halving bytes halves the DMA floor); PSUM accumulation
stays fp32, output is bf16 and upcast on host.

Per-core device kernel:
  stage 1: A in c2-major layout via 64 tiny matmuls (one per q):
           pa[c2, (s, sp, q)] += wt_q[u, c2]^T @ ui[u, b].  PE transposes
           (one per sample pair) then produce the block-diagonal stage-2
           lhsT with q on partitions; no DRAM bounce, tiny evacuations.
  stage 2: per sample-pair, block-diagonal style matmuls (K=128) + a K=3
           accumulating matmul folding V*t, bias and the "+1" of
           (1+gamma).
  norm:    bn_stats/bn_aggr per pair tile (2 samples x 64 ch = 128 rows);
           xm = (x - mean) * rstd on DVE (4x bf16 mode), params
           evacuated PSUM->bf16 on ACT, then out = xm*(1+gamma) + beta
           as two 2x-mode DVE tensor_tensor ops.
"""

import json

import numpy as np
import ml_dtypes

for _p in ("/opt/trn_rl_repo", "/root/.axon_site/_ro/trn_rl_repo"):
    import sys as _sys

    if _p not in _sys.path:
        _sys.path.append(_p)

import concourse.bass as bass
import concourse.mybir as mybir
from concourse.tile import TileContext
from concourse.bass_utils import run_bass_kernel_spmd
from concourse import masks


def _split_sync_waits(raw: bytes, keep: int = 1) -> bytes:
    """Walrus in this env accepts at most one sync wait per TPB instruction.

    Tile packs several waits into sync_info.on_wait; re-emit the excess as
    standalone single-wait EventSemaphore instructions (what wait_ge emits)
    immediately before the instruction, in the same engine stream.
    """
    bir = json.loads(raw)
    n = 0
    for fn in bir["functions"]:
        for blk in fn["blocks"]:
            out = []
            for ins in blk["instructions"]:
                si = ins.get("sync_info")
                ws = si.get("on_wait") if si else None
                if ws and len(ws) > keep:
                    for w in ws[: len(ws) - keep]:
                        n += 1
                        out.append(
                            {
                                "debug": ins.get("debug", 0),
                                "engine": ins["engine"],
                                "ins": [],
                                "outs": [],
                                "name": f"evw-{n}",
                                "opcode": "EventSemaphore",
                                "sync_info": {"on_update": [], "on_wait": [w]},
                            }
                        )
                    si["on_wait"] = ws[len(ws) - keep :]
                out.append(ins)
            blk["instructions"] = out
    return json.dumps(bir).encode()


class _Bass(bass.Bass):
    def to_json_bytes(self) -> bytes:
        return _split_sync_waits(super().to_json_bytes())


B, C, L = 16, 256, 1024
U, Q = 64, 64
EPS = 1e-5
N_CORES = 8
BG, CG = 2, 4          # batch groups x channel groups
BPC = B // BG          # samples per core = 8
CPC = C // CG          # channels per core = 64
NPAIR = BPC // 2       # sample pairs per core = 4

FP32 = mybir.dt.float32
BF16 = mybir.dt.bfloat16

_CACHE = {}


def _build_nc(detect_races: bool = True):
    nc = _Bass(detect_race_conditions=detect_races)

    # x / e pair-group tiles: [g, 128 rows (2 samples x 64 ch/q), 2 pairs x L]
    xg_in = nc.dram_tensor("xg_s", [2, 128, 2 * L], BF16, kind="ExternalInput")
    eg_in = nc.dram_tensor("eg_s", [2, 128, 2 * L], BF16, kind="ExternalInput")
    # wt: stage-1 lhsT per q: wt[u, q*128 + c2] = W2[c2, u, q]
    wt_in = nc.dram_tensor("wt_s", [64, 64 * 128], BF16, kind="ExternalInput")
    ui_in = nc.dram_tensor("ui_s", [64, BPC], BF16, kind="ExternalInput")
    # sm: [r2 (4 pairs x 1024) | l2 (256)] on 3 partitions (fp32)
    sm_in = nc.dram_tensor("sm2", [3, NPAIR * L + 256], FP32, kind="ExternalInput")
    out_d = nc.dram_tensor("out_s", [2, 128, 2 * L], BF16, kind="ExternalOutput")

    AF = mybir.ActivationFunctionType
    OP = mybir.AluOpType
    F32R = mybir.dt.float32r

    with TileContext(nc) as tc:
        with (
            tc.tile_pool(name="persist", bufs=1) as persist,
            tc.tile_pool(name="stat", bufs=8) as stat,
            tc.tile_pool(name="work", bufs=4) as work,
            tc.tile_pool(name="psM", bufs=4, space="PSUM") as psM,
        ):
            # small local tiles first (no DMA deps) so PE warm-up can start
            dum = persist.tile([64, 512], BF16, tag="dum")
            nc.vector.memset(dum, 0.0)
            eps_t = persist.tile([128, 1], FP32, tag="eps")
            nc.vector.memset(eps_t, EPS)
            idn = persist.tile([128, 128], BF16, tag="idn")
            masks.make_identity(nc, idn[:, :])
            # block-diagonal stage-2 lhsT, zero blocks stay zero
            lt = persist.tile([128, NPAIR, 256], BF16, tag="lt")
            nc.gpsimd.memset(lt[:, :, :], 0.0)

            # ---- input DMAs (DMA device is serialized; order matters):
            # ui (tiny) -> x group 0 (unblocks stats) -> wt (unblocks the
            # stage-1 -> lt chain) -> e group 0 + sm (unblocks stage 2) ->
            # x/e group 1.
            sm = persist.tile([3, NPAIR * L + 256], F32R, tag="sm")
            nc.sync.dma_start(out=sm, in_=sm_in[:, :].bitcast(F32R))
            r2 = sm[:, 0 : NPAIR * L].rearrange("k (s l) -> k s l", s=NPAIR)
            l2 = sm[:, NPAIR * L : NPAIR * L + 256]
            uit = persist.tile([64, BPC], BF16, tag="uit")
            nc.sync.dma_start(out=uit, in_=ui_in[:, :])
            xgt = persist.tile([128, 2, 2 * L], BF16, tag="xgt")
            egt = persist.tile([128, 2, 2 * L], BF16, tag="egt")
            nc.sync.dma_start(out=xgt[:, 0, :], in_=xg_in[0, :, :])
            wt = persist.tile([64, 64 * 128], BF16, tag="wt")
            nc.sync.dma_start(out=wt[:, 0 : 32 * 128], in_=wt_in[:, 0 : 32 * 128])
            nc.sync.dma_start(out=wt[:, 32 * 128 :], in_=wt_in[:, 32 * 128 :])
            nc.sync.dma_start(out=egt[:, 0, :], in_=eg_in[0, :, :])
            nc.sync.dma_start(out=egt[:, 1, :], in_=eg_in[1, :, :])
            nc.sync.dma_start(out=xgt[:, 1, 0:L], in_=xg_in[1, :, 0:L])
            nc.sync.dma_start(out=xgt[:, 1, L:], in_=xg_in[1, :, L:])

            ott = persist.tile([128, 2, 2 * L], BF16, tag="ott")

            # ---- PE warm-up: start the p-state ramp clock early ----
            # pa cells are all rewritten with start=True by stage 1.
            pa = psM.tile([128, 4, 2, 64], FP32, tag="ps", name="pa")
            paw = pa.rearrange("p s h q -> p (s h q)")
            for wu in range(3):
                nc.tensor.matmul(
                    paw[0:8, 0:512],
                    lhsT=dum[:, 0:8],
                    rhs=dum[:, 0:512],
                    start=True,
                    stop=True,
                )

            # ---- stage 1: pa[c2, (s, sp, q)] = sum_u wt_q[u, c2] ui[u, b] ----
            # b = 2s + sp; out free dims (s:4, sp:2) are strided, offset q.
            a_sb = persist.tile([128, 512], BF16, tag="a_sb")
            asv = a_sb.rearrange("p (s h q) -> p s h q", s=NPAIR, h=2)
            for qh in range(2):
                for q in range(qh * 32, qh * 32 + 32):
                    nc.tensor.matmul(
                        pa[:, :, :, q],
                        lhsT=wt[:, q * 128 : (q + 1) * 128],
                        rhs=uit.rearrange("u (s h) -> u s h", s=NPAIR),
                        start=True,
                        stop=True,
                    )
                # evacuate A half to SBUF bf16, layout [c2, (s, sp*64+q)]
                nc.scalar.activation(
                    out=asv[:, :, :, qh * 32 : qh * 32 + 32],
                    in_=pa[:, :, :, qh * 32 : qh * 32 + 32],
                    func=AF.Copy,
                )

            # per pair: PE transpose [c2, (sp,q)] -> [(sp,q), c2] (bf16 PSUM)
            # same tag as pa: reuses its bank once evac'd
            pt = psM.tile([128, 4, 128], BF16, tag="ps", name="pt")
            for s in range(NPAIR):
                nc.tensor.transpose(
                    pt[:, s, :], a_sb[:, s * 128 : (s + 1) * 128], idn[:, :]
                )
            # scatter into the block-diagonal lhsT:
            #   lt[sp*64+q, s, gb*128 + sp*64 + c'] = A[2s+sp, gb*64+c', q]
            lt4 = lt.rearrange("p s (gb c) -> p s gb c", gb=2)
            pt4 = pt.rearrange("p s (gb c) -> p s gb c", gb=2)
            for s in range(NPAIR):
                for sp in range(2):
                    rows = slice(sp * 64, sp * 64 + 64)
                    nc.scalar.activation(
                        out=lt4[rows, s, :, sp * 64 : sp * 64 + 64],
                        in_=pt4[rows, s, :, :],
                        func=AF.Copy,
                    )

            # ---- norm stats for all pairs first (keeps the DVE queue
            # free of combine ops that wait on ACT evacuations) ----
            od = out_d.rearrange("g p (i l) -> g p i l", i=2)
            xms, mvs, rstds = [], [], []
            for s in range(NPAIR):
                g, i = divmod(s, 2)
                xt = xgt[:, g, i * L : (i + 1) * L]
                st = stat.tile([128, 2, 6], FP32, tag="st")
                nc.vector.bn_stats(st[:, 0, :], xt[:, 0:512])
                nc.vector.bn_stats(st[:, 1, :], xt[:, 512:1024])
                mv = stat.tile([128, 2], FP32, tag="mv")
                nc.vector.bn_aggr(mv, st)
                rstd = stat.tile([128, 1], FP32, tag="rstd")
                nc.scalar.activation(
                    out=rstd, in_=mv[:, 1:2], func=AF.Sqrt, bias=eps_t, scale=1.0
                )
                nc.vector.reciprocal(rstd, rstd)
                xm = work.tile([128, L], BF16, tag="xm", name=f"xm{s}")
                nc.vector.tensor_scalar(
                    out=xm,
                    in0=xt,
                    scalar1=mv[:, 0:1],
                    scalar2=rstd,
                    op0=OP.subtract,
                    op1=OP.mult,
                )
                xms.append(xm)

            # ---- stage 2 matmuls per pair ----
            pms, pbs = [], []
            for s in range(NPAIR):
                g, i = divmod(s, 2)
                et = egt[:, g, i * L : (i + 1) * L]
                pm = psM.tile([128, L], FP32, tag="ps", name=f"pm{s}")
                pb = psM.tile([128, L], FP32, tag="ps", name=f"pb{s}")
                pms.append(pm)
                pbs.append(pb)
                for h in range(2):
                    cols = slice(h * 512, (h + 1) * 512)
                    nc.tensor.matmul(
                        pm[:, cols], lhsT=lt[:, s, 0:128], rhs=et[:, cols],
                        start=True, stop=False,
                    )
                    nc.tensor.matmul(
                        pm[:, cols], lhsT=l2[:, 0:128], rhs=r2[:, s, cols],
                        start=False, stop=True,
                    )
                    nc.tensor.matmul(
                        pb[:, cols], lhsT=lt[:, s, 128:256], rhs=et[:, cols],
                        start=True, stop=False,
                    )
                    nc.tensor.matmul(
                        pb[:, cols], lhsT=l2[:, 128:256], rhs=r2[:, s, cols],
                        start=False, stop=True,
                    )
                # ACT evacuations for pairs 0-2 as soon as params land
                if s < NPAIR - 1:
                    mg = work.tile([128, L], BF16, tag="mg", name=f"mg{s}")
                    nc.scalar.activation(out=mg, in_=pm, func=AF.Copy)
                    mb = work.tile([128, L], BF16, tag="mb", name=f"mb{s}")
                    nc.scalar.activation(out=mb, in_=pb, func=AF.Copy)
                    pms[s], pbs[s] = mg, mb

            # ---- combines: ot = xm * (1+gamma) + beta; pair 3 goes
            # DVE-direct from PSUM (its ACT evac would queue last), and
            # runs before pair 2 whose evacuations finish last on ACT ----
            for s in [0, 1, 3, 2]:
                g, i = divmod(s, 2)
                otv = ott[:, g, i * L : (i + 1) * L]
                if s < NPAIR - 1:
                    nc.vector.tensor_tensor(out=otv, in0=xms[s], in1=pms[s], op=OP.mult)
                    nc.vector.tensor_tensor(out=otv, in0=otv, in1=pbs[s], op=OP.add)
                else:
                    prd = work.tile([128, L], BF16, tag="mg", name="prd3")
                    nc.vector.scalar_tensor_tensor(
                        out=prd, in0=pms[s], scalar=1.0, in1=xms[s],
                        op0=OP.mult, op1=OP.mult,
                    )
                    nc.vector.tensor_tensor(out=otv, in0=prd, in1=pbs[s], op=OP.add)
                nc.sync.dma_start(out=od[g, :, i, :], in_=otv)

    return nc


def _prep_core_inputs(core, x, u_i, e_qid, t, W, V, bias):
    bg, cg = divmod(core, CG)
    bs = slice(bg * BPC, (bg + 1) * BPC)
    rg = slice(cg * CPC, (cg + 1) * CPC)
    rb = slice(C + cg * CPC, C + (cg + 1) * CPC)
    bf = ml_dtypes.bfloat16

    # x / e pair tiles -> groups of 2 pairs side by side
    xp = x[bs, rg, :].reshape(NPAIR, 128, L)
    ep = e_qid[bs].reshape(NPAIR, 128, L)
    xg = np.concatenate([xp[0::2], xp[1::2]], axis=2)   # [2, 128, 2L]
    eg = np.concatenate([ep[0::2], ep[1::2]], axis=2)

    w2 = np.concatenate([W[rg], W[rb]], axis=0)          # (128, 4096) c2=[g|b]
    wr = w2.reshape(128, U, Q)                           # [c2, u, q]
    wt = np.ascontiguousarray(wr.transpose(1, 2, 0)).reshape(64, Q * 128)

    ui_s = np.ascontiguousarray(u_i[bs].T)               # (64, 8)

    vg, vb = V[rg, 0], V[rb, 0]
    bgm, bbt = bias[rg], bias[rb]
    l2 = np.zeros((3, 256), np.float32)
    l2[0, 0:64] = vg
    l2[1, 64:128] = vg
    l2[2, 0:64] = 1.0 + bgm
    l2[2, 64:128] = 1.0 + bgm
    l2[0, 128:192] = vb
    l2[1, 192:256] = vb
    l2[2, 128:192] = bbt
    l2[2, 192:256] = bbt

    r2 = np.empty((3, NPAIR, L), np.float32)
    for s in range(NPAIR):
        r2[0, s] = t[bg * BPC + 2 * s, 0]
        r2[1, s] = t[bg * BPC + 2 * s + 1, 0]
    r2[2] = 1.0
    sm = np.concatenate([r2.reshape(3, NPAIR * L), l2], axis=1)

    return {
        "xg_s": np.ascontiguousarray(xg).astype(bf),
        "eg_s": np.ascontiguousarray(eg).astype(bf),
        "wt_s": wt.astype(bf),
        "ui_s": ui_s.astype(bf),
        "sm2": np.ascontiguousarray(sm, dtype=np.float32),
    }


def kernel(x, u_i, e_qid, t, W, V, bias):
    x = np.asarray(x, np.float32)
    u_i = np.asarray(u_i, np.float32)
    e_qid = np.asarray(e_qid, np.float32)
    t = np.asarray(t, np.float32)
    W = np.asarray(W, np.float32)
    V = np.asarray(V, np.float32)
    bias = np.asarray(bias, np.float32)

    if "nc" not in _CACHE:
        _CACHE["nc"] = _build_nc()
    nc = _CACHE["nc"]

    in_maps = [
        _prep_core_inputs(i, x, u_i, e_qid, t, W, V, bias) for i in range(N_CORES)
    ]
    results = run_bass_kernel_spmd(nc, in_maps, list(range(N_CORES))).results

    out = np.empty((B, C, L), np.float32)
    for i in range(N_CORES):
        bg, cg = divmod(i, CG)
        res = np.asarray(results[i]["out_s"], dtype=np.float32)  # [2, 128, 2L]
        # [g, (sp c), (i l)] -> sample b = 4g + 2i + sp
        res = res.reshape(2, 2, CPC, 2, L).transpose(0, 3, 1, 2, 4)
        out[bg * BPC : (bg + 1) * BPC, cg * CPC : (cg + 1) * CPC, :] = res.reshape(
            BPC, CPC, L
        )
    return out


# revision 11
# speedup vs baseline: 1.0148x; 1.0040x over previous
"""ConditionAwareAdaIN Trainium2 kernel (bf16 rewrite).

Reference computation (B=16, C=256, L=1024, U=64, Q=64):
    nx    = InstanceNorm1d(x)                       # per-(b,c) stats over L
    A     = einsum('bu,cuq->bcq', u_i, W.reshape(2C,U,Q))
    style = einsum('bcq,bql->bcl', A, e_qid)
    gamma, beta = split(style + V@t + bias, 2, axis=1)
    out   = (1 + gamma) * nx + beta

Sharding: 2-way over batch x 4-way over channels -> 8 cores, each owning
8 samples x 64 channels (its slice of gamma AND beta rows of W/V/bias).

All bulk tensors move as bf16 (DMA time on the cost model is serialized
bytes/360GB/s, so halving bytes halves the DMA floor); PSUM accumulation
stays fp32, output is bf16 and upcast on host.

Per-core device kernel:
  stage 1: A in c2-major layout via 64 tiny matmuls (one per q):
           pa[c2, (s, sp, q)] += wt_q[u, c2]^T @ ui[u, b].  PE transposes
           (one per sample pair) then produce the block-diagonal stage-2
           lhsT with q on partitions; no DRAM bounce, tiny evacuations.
  stage 2: per sample-pair, block-diagonal style matmuls (K=128) + a K=3
           accumulating matmul folding V*t, bias and the "+1" of
           (1+gamma).
  norm:    bn_stats/bn_aggr per pair tile (2 samples x 64 ch = 128 rows);
           xm = (x - mean) * rstd on DVE (4x bf16 mode), params
           evacuated PSUM->bf16 on ACT, then out = xm*(1+gamma) + beta
           as two 2x-mode DVE tensor_tensor ops.
"""

import json

import numpy as np
import ml_dtypes

for _p in ("/opt/trn_rl_repo", "/root/.axon_site/_ro/trn_rl_repo"):
    import sys as _sys

    if _p not in _sys.path:
        _sys.path.append(_p)

import concourse.bass as bass
import concourse.mybir as mybir
from concourse.tile import TileContext
from concourse.bass_utils import run_bass_kernel_spmd
from concourse import masks


def _split_sync_waits(raw: bytes, keep: int = 1) -> bytes:
    """Walrus in this env accepts at most one sync wait per TPB instruction.

    Tile packs several waits into sync_info.on_wait; re-emit the excess as
    standalone single-wait EventSemaphore instructions (what wait_ge emits)
    immediately before the instruction, in the same engine stream.
    """
    bir = json.loads(raw)
    n = 0
    for fn in bir["functions"]:
        for blk in fn["blocks"]:
            out = []
            for ins in blk["instructions"]:
                si = ins.get("sync_info")
                ws = si.get("on_wait") if si else None
                if ws and len(ws) > keep:
                    for w in ws[: len(ws) - keep]:
                        n += 1
                        out.append(
                            {
                                "debug": ins.get("debug", 0),
                                "engine": ins["engine"],
                                "ins": [],
                                "outs": [],
                                "name": f"evw-{n}",
                                "opcode": "EventSemaphore",
                                "sync_info": {"on_update": [], "on_wait": [w]},
                            }
                        )
                    si["on_wait"] = ws[len(ws) - keep :]
                out.append(ins)
            blk["instructions"] = out
    return json.dumps(bir).encode()


class _Bass(bass.Bass):
    def to_json_bytes(self) -> bytes:
        return _split_sync_waits(super().to_json_bytes())


B, C, L = 16, 256, 1024
U, Q = 64, 64
EPS = 1e-5
N_CORES = 8
BG, CG = 2, 4          # batch groups x channel groups
BPC = B // BG          # samples per core = 8
CPC = C // CG          # channels per core = 64
NPAIR = BPC // 2       # sample pairs per core = 4

FP32 = mybir.dt.float32
BF16 = mybir.dt.bfloat16

_CACHE = {}


def _build_nc(detect_races: bool = True):
    nc = _Bass(detect_race_conditions=detect_races)

    # x / e pair-group tiles: [g, 128 rows (2 samples x 64 ch/q), 2 pairs x L]
    xg_in = nc.dram_tensor("xg_s", [2, 128, 2 * L], BF16, kind="ExternalInput")
    eg_in = nc.dram_tensor("eg_s", [2, 128, 2 * L], BF16, kind="ExternalInput")
    # wt: stage-1 lhsT per q: wt[u, q*128 + c2] = W2[c2, u, q]
    wt_in = nc.dram_tensor("wt_s", [64, 64 * 128], BF16, kind="ExternalInput")
    ui_in = nc.dram_tensor("ui_s", [64, BPC], BF16, kind="ExternalInput")
    # sm: [r2 (4 pairs x 1024) | l2 (256)] on 3 partitions (fp32)
    sm_in = nc.dram_tensor("sm2", [3, NPAIR * L + 256], FP32, kind="ExternalInput")
    out_d = nc.dram_tensor("out_s", [2, 128, 2 * L], BF16, kind="ExternalOutput")

    AF = mybir.ActivationFunctionType
    OP = mybir.AluOpType
    F32R = mybir.dt.float32r

    with TileContext(nc) as tc:
        with (
            tc.tile_pool(name="persist", bufs=1) as persist,
            tc.tile_pool(name="stat", bufs=8) as stat,
            tc.tile_pool(name="work", bufs=4) as work,
            tc.tile_pool(name="psM", bufs=4, space="PSUM") as psM,
        ):
            # small local tiles first (no DMA deps) so PE warm-up can start
            dum = persist.tile([64, 512], BF16, tag="dum")
            nc.vector.memset(dum, 0.0)
            eps_t = persist.tile([128, 1], FP32, tag="eps")
            nc.vector.memset(eps_t, EPS)
            idn = persist.tile([128, 128], BF16, tag="idn")
            masks.make_identity(nc, idn[:, :])
            # block-diagonal stage-2 lhsT, zero blocks stay zero
            lt = persist.tile([128, NPAIR, 256], BF16, tag="lt")
            nc.gpsimd.memset(lt[:, :, :], 0.0)

            # ---- input DMAs (DMA device is serialized; order matters):
            # ui (tiny) -> x group 0 (unblocks stats) -> wt (unblocks the
            # stage-1 -> lt chain) -> e group 0 + sm (unblocks stage 2) ->
            # x/e group 1.
            sm = persist.tile([3, NPAIR * L + 256], F32R, tag="sm")
            nc.sync.dma_start(out=sm, in_=sm_in[:, :].bitcast(F32R))
            r2 = sm[:, 0 : NPAIR * L].rearrange("k (s l) -> k s l", s=NPAIR)
            l2 = sm[:, NPAIR * L : NPAIR * L + 256]
            uit = persist.tile([64, BPC], BF16, tag="uit")
            nc.sync.dma_start(out=uit, in_=ui_in[:, :])
            xgt = persist.tile([128, 2, 2 * L], BF16, tag="xgt")
            egt = persist.tile([128, 2, 2 * L], BF16, tag="egt")
            nc.sync.dma_start(out=xgt[:, 0, :], in_=xg_in[0, :, :])
            wt = persist.tile([64, 64 * 128], BF16, tag="wt")
            nc.sync.dma_start(out=wt[:, 0 : 32 * 128], in_=wt_in[:, 0 : 32 * 128])
            nc.sync.dma_start(out=wt[:, 32 * 128 :], in_=wt_in[:, 32 * 128 :])
            nc.sync.dma_start(out=egt[:, 0, :], in_=eg_in[0, :, :])
            nc.sync.dma_start(out=egt[:, 1, :], in_=eg_in[1, :, :])
            nc.sync.dma_start(out=xgt[:, 1, 0:L], in_=xg_in[1, :, 0:L])
            nc.sync.dma_start(out=xgt[:, 1, L:], in_=xg_in[1, :, L:])

            ott = persist.tile([128, 2, 2 * L], BF16, tag="ott")

            # ---- PE warm-up: start the p-state ramp clock early ----
            # pa cells are all rewritten with start=True by stage 1.
            pa = psM.tile([128, 4, 2, 64], FP32, tag="ps", name="pa")
            paw = pa.rearrange("p s h q -> p (s h q)")
            for wu in range(3):
                nc.tensor.matmul(
                    paw[0:8, 0:512],
                    lhsT=dum[:, 0:8],
                    rhs=dum[:, 0:512],
                    start=True,
                    stop=True,
                )

            # ---- stage 1: pa[c2, (s, sp, q)] = sum_u wt_q[u, c2] ui[u, b] ----
            # b = 2s + sp; out free dims (s:4, sp:2) are strided, offset q.
            a_sb = persist.tile([128, 512], BF16, tag="a_sb")
            asv = a_sb.rearrange("p (s h q) -> p s h q", s=NPAIR, h=2)
            for qh in range(2):
                for q in range(qh * 32, qh * 32 + 32):
                    nc.tensor.matmul(
                        pa[:, :, :, q],
                        lhsT=wt[:, q * 128 : (q + 1) * 128],
                        rhs=uit.rearrange("u (s h) -> u s h", s=NPAIR),
                        start=True,
                        stop=True,
                    )
                # evacuate A half to SBUF bf16, layout [c2, (s, sp*64+q)]
                nc.scalar.activation(
                    out=asv[:, :, :, qh * 32 : qh * 32 + 32],
                    in_=pa[:, :, :, qh * 32 : qh * 32 + 32],
                    func=AF.Copy,
                )

            # per pair: PE transpose [c2, (sp,q)] -> [(sp,q), c2] (bf16 PSUM)
            # same tag as pa: reuses its bank once evac'd
            pt = psM.tile([128, 4, 128], BF16, tag="ps", name="pt")
            for s in range(NPAIR):
                nc.tensor.transpose(
                    pt[:, s, :], a_sb[:, s * 128 : (s + 1) * 128], idn[:, :]
                )
            # scatter into the block-diagonal lhsT:
            #   lt[sp*64+q, s, gb*128 + sp*64 + c'] = A[2s+sp, gb*64+c', q]
            lt4 = lt.rearrange("p s (gb c) -> p s gb c", gb=2)
            pt4 = pt.rearrange("p s (gb c) -> p s gb c", gb=2)
            # (scatter into lt happens on DVE, emitted inside the stats
            # loop below so it lands in the DVE queue's idle gap between
            # the group-0 and DMA-gated group-1 stats)

            # ---- norm stats for all pairs first (keeps the DVE queue
            # free of combine ops that wait on ACT evacuations) ----
            od = out_d.rearrange("g p (i l) -> g p i l", i=2)
            xms, mvs, rstds = [], [], []
            for s in range(NPAIR):
                g, i = divmod(s, 2)
                xt = xgt[:, g, i * L : (i + 1) * L]
                st = stat.tile([128, 2, 6], FP32, tag="st")
                nc.vector.bn_stats(st[:, 0, :], xt[:, 0:512])
                nc.vector.bn_stats(st[:, 1, :], xt[:, 512:1024])
                mv = stat.tile([128, 2], FP32, tag="mv")
                nc.vector.bn_aggr(mv, st)
                rstd = stat.tile([128, 1], FP32, tag="rstd")
                nc.scalar.activation(
                    out=rstd, in_=mv[:, 1:2], func=AF.Sqrt, bias=eps_t, scale=1.0
                )
                nc.vector.reciprocal(rstd, rstd)
                xm = work.tile([128, L], BF16, tag="xm", name=f"xm{s}")
                nc.vector.tensor_scalar(
                    out=xm,
                    in0=xt,
                    scalar1=mv[:, 0:1],
                    scalar2=rstd,
                    op0=OP.subtract,
                    op1=OP.mult,
                )
                xms.append(xm)
                if s == 1:
                    for sp in range(2):
                        rows = slice(sp * 64, sp * 64 + 64)
                        nc.vector.tensor_copy(
                            out=lt4[rows, :, :, sp * 64 : sp * 64 + 64],
                            in_=pt4[rows, :, :, :],
                        )

            # ---- stage 2 matmuls per pair ----
            pms, pbs = [], []
            for s in range(NPAIR):
                g, i = divmod(s, 2)
                et = egt[:, g, i * L : (i + 1) * L]
                pm = psM.tile([128, L], FP32, tag="ps", name=f"pm{s}")
                pb = psM.tile([128, L], FP32, tag="ps", name=f"pb{s}")
                pms.append(pm)
                pbs.append(pb)
                for h in range(2):
                    cols = slice(h * 512, (h + 1) * 512)
                    nc.tensor.matmul(
                        pm[:, cols], lhsT=lt[:, s, 0:128], rhs=et[:, cols],
                        start=True, stop=False,
                    )
                    nc.tensor.matmul(
                        pm[:, cols], lhsT=l2[:, 0:128], rhs=r2[:, s, cols],
                        start=False, stop=True,
                    )
                    nc.tensor.matmul(
                        pb[:, cols], lhsT=lt[:, s, 128:256], rhs=et[:, cols],
                        start=True, stop=False,
                    )
                    nc.tensor.matmul(
                        pb[:, cols], lhsT=l2[:, 128:256], rhs=r2[:, s, cols],
                        start=False, stop=True,
                    )
                # ACT evacuations for pairs 0-2 as soon as params land
                if s < NPAIR - 1:
                    mg = work.tile([128, L], BF16, tag="mg", name=f"mg{s}")
                    nc.scalar.activation(out=mg, in_=pm, func=AF.Copy)
                    mb = work.tile([128, L], BF16, tag="mb", name=f"mb{s}")
                    nc.scalar.activation(out=mb, in_=pb, func=AF.Copy)
                    pms[s], pbs[s] = mg, mb

            # ---- combines: ot = xm * (1+gamma) + beta; pair 3 goes
            # DVE-direct from PSUM (its ACT evac would queue last), and
            # runs before pair 2 whose evacuations finish last on ACT ----
            for s in [0, 1, 3, 2]:
                g, i = divmod(s, 2)
                otv = ott[:, g, i * L : (i + 1) * L]
                if s < NPAIR - 1:
                    nc.vector.tensor_tensor(out=otv, in0=xms[s], in1=pms[s], op=OP.mult)
                    nc.vector.tensor_tensor(out=otv, in0=otv, in1=pbs[s], op=OP.add)
                else:
                    prd = work.tile([128, L], BF16, tag="mg", name="prd3")
                    nc.vector.scalar_tensor_tensor(
                        out=prd, in0=pms[s], scalar=1.0, in1=xms[s],
                        op0=OP.mult, op1=OP.mult,
                    )
                    nc.vector.tensor_tensor(out=otv, in0=prd, in1=pbs[s], op=OP.add)
                nc.sync.dma_start(out=od[g, :, i, :], in_=otv)

    return nc


def _prep_core_inputs(core, x, u_i, e_qid, t, W, V, bias):
    bg, cg = divmod(core, CG)
    bs = slice(bg * BPC, (bg + 1) * BPC)
    rg = slice(cg * CPC, (cg + 1) * CPC)
    rb = slice(C + cg * CPC, C + (cg + 1) * CPC)
    bf = ml_dtypes.bfloat16

    # x / e pair tiles -> groups of 2 pairs side by side
    xp = x[bs, rg, :].reshape(NPAIR, 128, L)
    ep = e_qid[bs].reshape(NPAIR, 128, L)
    xg = np.concatenate([xp[0::2], xp[1::2]], axis=2)   # [2, 128, 2L]
    eg = np.concatenate([ep[0::2], ep[1::2]], axis=2)

    w2 = np.concatenate([W[rg], W[rb]], axis=0)          # (128, 4096) c2=[g|b]
    wr = w2.reshape(128, U, Q)                           # [c2, u, q]
    wt = np.ascontiguousarray(wr.transpose(1, 2, 0)).reshape(64, Q * 128)

    ui_s = np.ascontiguousarray(u_i[bs].T)               # (64, 8)

    vg, vb = V[rg, 0], V[rb, 0]
    bgm, bbt = bias[rg], bias[rb]
    l2 = np.zeros((3, 256), np.float32)
    l2[0, 0:64] = vg
    l2[1, 64:128] = vg
    l2[2, 0:64] = 1.0 + bgm
    l2[2, 64:128] = 1.0 + bgm
    l2[0, 128:192] = vb
    l2[1, 192:256] = vb
    l2[2, 128:192] = bbt
    l2[2, 192:256] = bbt

    r2 = np.empty((3, NPAIR, L), np.float32)
    for s in range(NPAIR):
        r2[0, s] = t[bg * BPC + 2 * s, 0]
        r2[1, s] = t[bg * BPC + 2 * s + 1, 0]
    r2[2] = 1.0
    sm = np.concatenate([r2.reshape(3, NPAIR * L), l2], axis=1)

    return {
        "xg_s": np.ascontiguousarray(xg).astype(bf),
        "eg_s": np.ascontiguousarray(eg).astype(bf),
        "wt_s": wt.astype(bf),
        "ui_s": ui_s.astype(bf),
        "sm2": np.ascontiguousarray(sm, dtype=np.float32),
    }


def kernel(x, u_i, e_qid, t, W, V, bias):
    x = np.asarray(x, np.float32)
    u_i = np.asarray(u_i, np.float32)
    e_qid = np.asarray(e_qid, np.float32)
    t = np.asarray(t, np.float32)
    W = np.asarray(W, np.float32)
    V = np.asarray(V, np.float32)
    bias = np.asarray(bias, np.float32)

    if "nc" not in _CACHE:
        _CACHE["nc"] = _build_nc()
    nc = _CACHE["nc"]

    in_maps = [
        _prep_core_inputs(i, x, u_i, e_qid, t, W, V, bias) for i in range(N_CORES)
    ]
    results = run_bass_kernel_spmd(nc, in_maps, list(range(N_CORES))).results

    out = np.empty((B, C, L), np.float32)
    for i in range(N_CORES):
        bg, cg = divmod(i, CG)
        res = np.asarray(results[i]["out_s"], dtype=np.float32)  # [2, 128, 2L]
        # [g, (sp c), (i l)] -> sample b = 4g + 2i + sp
        res = res.reshape(2, 2, CPC, 2, L).transpose(0, 3, 1, 2, 4)
        out[bg * BPC : (bg + 1) * BPC, cg * CPC : (cg + 1) * CPC, :] = res.reshape(
            BPC, CPC, L
        )
    return out


# revision 12
# speedup vs baseline: 1.0354x; 1.0203x over previous
"""ConditionAwareAdaIN Trainium2 kernel (bf16 rewrite).

Reference computation (B=16, C=256, L=1024, U=64, Q=64):
    nx    = InstanceNorm1d(x)                       # per-(b,c) stats over L
    A     = einsum('bu,cuq->bcq', u_i, W.reshape(2C,U,Q))
    style = einsum('bcq,bql->bcl', A, e_qid)
    gamma, beta = split(style + V@t + bias, 2, axis=1)
    out   = (1 + gamma) * nx + beta

Sharding: 2-way over batch x 4-way over channels -> 8 cores, each owning
8 samples x 64 channels (its slice of gamma AND beta rows of W/V/bias).

All bulk tensors move as bf16 (DMA time on the cost model is serialized
bytes/360GB/s, so halving bytes halves the DMA floor); PSUM accumulation
stays fp32, output is bf16 and upcast on host.

Per-core device kernel:
  stage 1: A in c2-major layout via 64 tiny matmuls (one per q):
           pa[c2, (s, sp, q)] += wt_q[u, c2]^T @ ui[u, b].  PE transposes
           (one per sample pair) then produce the block-diagonal stage-2
           lhsT with q on partitions; no DRAM bounce, tiny evacuations.
  stage 2: per sample-pair, block-diagonal style matmuls (K=128) + a K=3
           accumulating matmul folding V*t, bias and the "+1" of
           (1+gamma).
  norm:    bn_stats/bn_aggr per pair tile (2 samples x 64 ch = 128 rows);
           xm = (x - mean) * rstd on DVE (4x bf16 mode), params
           evacuated PSUM->bf16 on ACT, then out = xm*(1+gamma) + beta
           as two 2x-mode DVE tensor_tensor ops.
"""

import json

import numpy as np
import ml_dtypes

for _p in ("/opt/trn_rl_repo", "/root/.axon_site/_ro/trn_rl_repo"):
    import sys as _sys

    if _p not in _sys.path:
        _sys.path.append(_p)

import concourse.bass as bass
import concourse.mybir as mybir
from concourse.tile import TileContext
from concourse.bass_utils import run_bass_kernel_spmd
from concourse import masks


def _split_sync_waits(raw: bytes, keep: int = 1) -> bytes:
    """Walrus in this env accepts at most one sync wait per TPB instruction.

    Tile packs several waits into sync_info.on_wait; re-emit the excess as
    standalone single-wait EventSemaphore instructions (what wait_ge emits)
    immediately before the instruction, in the same engine stream.
    """
    bir = json.loads(raw)
    n = 0
    for fn in bir["functions"]:
        for blk in fn["blocks"]:
            out = []
            for ins in blk["instructions"]:
                si = ins.get("sync_info")
                ws = si.get("on_wait") if si else None
                if ws and len(ws) > keep:
                    for w in ws[: len(ws) - keep]:
                        n += 1
                        out.append(
                            {
                                "debug": ins.get("debug", 0),
                                "engine": ins["engine"],
                                "ins": [],
                                "outs": [],
                                "name": f"evw-{n}",
                                "opcode": "EventSemaphore",
                                "sync_info": {"on_update": [], "on_wait": [w]},
                            }
                        )
                    si["on_wait"] = ws[len(ws) - keep :]
                out.append(ins)
            blk["instructions"] = out
    return json.dumps(bir).encode()


class _Bass(bass.Bass):
    def to_json_bytes(self) -> bytes:
        return _split_sync_waits(super().to_json_bytes())


B, C, L = 16, 256, 1024
U, Q = 64, 64
EPS = 1e-5
N_CORES = 8
BG, CG = 2, 4          # batch groups x channel groups
BPC = B // BG          # samples per core = 8
CPC = C // CG          # channels per core = 64
NPAIR = BPC // 2       # sample pairs per core = 4

FP32 = mybir.dt.float32
BF16 = mybir.dt.bfloat16

_CACHE = {}


def _build_nc(detect_races: bool = True):
    nc = _Bass(detect_race_conditions=detect_races)

    # x / e pair-group tiles: [g, 128 rows (2 samples x 64 ch/q), 2 pairs x L]
    xg_in = nc.dram_tensor("xg_s", [2, 128, 2 * L], BF16, kind="ExternalInput")
    eg_in = nc.dram_tensor("eg_s", [2, 128, 2 * L], BF16, kind="ExternalInput")
    # wt: stage-1 lhsT per q: wt[u, q*128 + c2] = W2[c2, u, q]
    wt_in = nc.dram_tensor("wt_s", [64, 64 * 128], BF16, kind="ExternalInput")
    ui_in = nc.dram_tensor("ui_s", [64, BPC], BF16, kind="ExternalInput")
    # sm: [r2 (4 pairs x 1024) | l2 (256)] on 3 partitions (fp32)
    sm_in = nc.dram_tensor("sm2", [3, NPAIR * L + 256], FP32, kind="ExternalInput")
    out_d = nc.dram_tensor("out_s", [2, 128, 2 * L], BF16, kind="ExternalOutput")

    AF = mybir.ActivationFunctionType
    OP = mybir.AluOpType
    F32R = mybir.dt.float32r

    with TileContext(nc) as tc:
        with (
            tc.tile_pool(name="persist", bufs=1) as persist,
            tc.tile_pool(name="stat", bufs=8) as stat,
            tc.tile_pool(name="work", bufs=4) as work,
            tc.tile_pool(name="psM", bufs=4, space="PSUM") as psM,
        ):
            # small local tiles first (no DMA deps) so PE warm-up can start
            dum = persist.tile([64, 512], BF16, tag="dum")
            nc.vector.memset(dum, 0.0)
            eps_t = persist.tile([128, 1], FP32, tag="eps")
            nc.vector.memset(eps_t, EPS)
            idn = persist.tile([128, 128], BF16, tag="idn")
            masks.make_identity(nc, idn[:, :])
            # block-diagonal stage-2 lhsT, zero blocks stay zero
            lt = persist.tile([128, NPAIR, 256], BF16, tag="lt")
            nc.gpsimd.memset(lt[:, :, :], 0.0)

            # ---- input DMAs (DMA device is serialized; order matters):
            # ui (tiny) -> x group 0 (unblocks stats) -> wt (unblocks the
            # stage-1 -> lt chain) -> e group 0 + sm (unblocks stage 2) ->
            # x/e group 1.
            sm = persist.tile([3, NPAIR * L + 256], F32R, tag="sm")
            nc.sync.dma_start(out=sm, in_=sm_in[:, :].bitcast(F32R))
            r2 = sm[:, 0 : NPAIR * L].rearrange("k (s l) -> k s l", s=NPAIR)
            l2 = sm[:, NPAIR * L : NPAIR * L + 256]
            uit = persist.tile([64, BPC], BF16, tag="uit")
            nc.sync.dma_start(out=uit, in_=ui_in[:, :])
            xgt = persist.tile([128, 2, 2 * L], BF16, tag="xgt")
            egt = persist.tile([128, 2, 2 * L], BF16, tag="egt")
            nc.sync.dma_start(out=xgt[:, 0, :], in_=xg_in[0, :, :])
            wt = persist.tile([64, 64 * 128], BF16, tag="wt")
            nc.sync.dma_start(out=wt[:, 0 : 32 * 128], in_=wt_in[:, 0 : 32 * 128])
            nc.sync.dma_start(out=wt[:, 32 * 128 :], in_=wt_in[:, 32 * 128 :])
            nc.sync.dma_start(out=egt[:, 0, :], in_=eg_in[0, :, :])
            nc.sync.dma_start(out=egt[:, 1, :], in_=eg_in[1, :, :])
            nc.sync.dma_start(out=xgt[:, 1, 0:L], in_=xg_in[1, :, 0:L])
            nc.sync.dma_start(out=xgt[:, 1, L:], in_=xg_in[1, :, L:])

            ott = persist.tile([128, 2, 2 * L], BF16, tag="ott")

            # ---- PE warm-up: start the p-state ramp clock early ----
            # pa cells are all rewritten with start=True by stage 1.
            pa = psM.tile([128, 4, 2, 64], FP32, tag="ps", name="pa")
            paw = pa.rearrange("p s h q -> p (s h q)")
            for wu in range(3):
                nc.tensor.matmul(
                    paw[0:8, 0:512],
                    lhsT=dum[:, 0:8],
                    rhs=dum[:, 0:512],
                    start=True,
                    stop=True,
                )

            # ---- stage 1: pa[c2, (s, sp, q)] = sum_u wt_q[u, c2] ui[u, b] ----
            # b = 2s + sp; out free dims (s:4, sp:2) are strided, offset q.
            a_sb = persist.tile([128, 512], BF16, tag="a_sb")
            asv = a_sb.rearrange("p (s h q) -> p s h q", s=NPAIR, h=2)
            for qh in range(2):
                for q in range(qh * 32, qh * 32 + 32):
                    nc.tensor.matmul(
                        pa[:, :, :, q],
                        lhsT=wt[:, q * 128 : (q + 1) * 128],
                        rhs=uit.rearrange("u (s h) -> u s h", s=NPAIR),
                        start=True,
                        stop=True,
                    )
                # evacuate A half to SBUF bf16, layout [c2, (s, sp*64+q)]
                nc.scalar.activation(
                    out=asv[:, :, :, qh * 32 : qh * 32 + 32],
                    in_=pa[:, :, :, qh * 32 : qh * 32 + 32],
                    func=AF.Copy,
                )

            # per pair: PE transpose [c2, (sp,q)] -> [(sp,q), c2] (bf16 PSUM)
            # same tag as pa: reuses its bank once evac'd
            pt = psM.tile([128, 4, 128], BF16, tag="ps", name="pt")
            for s in range(NPAIR):
                nc.tensor.transpose(
                    pt[:, s, :], a_sb[:, s * 128 : (s + 1) * 128], idn[:, :]
                )
            # scatter into the block-diagonal lhsT:
            #   lt[sp*64+q, s, gb*128 + sp*64 + c'] = A[2s+sp, gb*64+c', q]
            lt4 = lt.rearrange("p s (gb c) -> p s gb c", gb=2)
            pt4 = pt.rearrange("p s (gb c) -> p s gb c", gb=2)
            # (scatter into lt happens on DVE, emitted inside the stats
            # loop below so it lands in the DVE queue's idle gap between
            # the group-0 and DMA-gated group-1 stats)

            # ---- norm stats for all pairs first (keeps the DVE queue
            # free of combine ops that wait on ACT evacuations) ----
            od = out_d.rearrange("g p (i l) -> g p i l", i=2)
            xms, mvs, rstds = [], [], []
            for s in range(NPAIR):
                g, i = divmod(s, 2)
                xt = xgt[:, g, i * L : (i + 1) * L]
                st = stat.tile([128, 2, 6], FP32, tag="st")
                nc.vector.bn_stats(st[:, 0, :], xt[:, 0:512])
                nc.vector.bn_stats(st[:, 1, :], xt[:, 512:1024])
                mv = stat.tile([128, 2], FP32, tag="mv")
                nc.vector.bn_aggr(mv, st)
                rstd = stat.tile([128, 1], FP32, tag="rstd")
                nc.scalar.activation(
                    out=rstd, in_=mv[:, 1:2], func=AF.Sqrt, bias=eps_t, scale=1.0
                )
                nc.vector.reciprocal(rstd, rstd)
                xm = work.tile([128, L], BF16, tag="xm", name=f"xm{s}")
                nc.vector.tensor_scalar(
                    out=xm,
                    in0=xt,
                    scalar1=mv[:, 0:1],
                    scalar2=rstd,
                    op0=OP.subtract,
                    op1=OP.mult,
                )
                xms.append(xm)
                if s == 1:
                    for sp in range(2):
                        rows = slice(sp * 64, sp * 64 + 64)
                        nc.vector.tensor_copy(
                            out=lt4[rows, :, :, sp * 64 : sp * 64 + 64],
                            in_=pt4[rows, :, :, :],
                        )

            # ---- stage 2 matmuls per pair ----
            pms, pbs = [], []
            for s in range(NPAIR):
                g, i = divmod(s, 2)
                et = egt[:, g, i * L : (i + 1) * L]
                pm = psM.tile([128, L], FP32, tag="ps", name=f"pm{s}")
                pb = psM.tile([128, L], FP32, tag="ps", name=f"pb{s}")
                pms.append(pm)
                pbs.append(pb)
                for h in range(2):
                    cols = slice(h * 512, (h + 1) * 512)
                    nc.tensor.matmul(
                        pm[:, cols], lhsT=lt[:, s, 0:128], rhs=et[:, cols],
                        start=True, stop=False,
                    )
                    nc.tensor.matmul(
                        pm[:, cols], lhsT=l2[:, 0:128], rhs=r2[:, s, cols],
                        start=False, stop=True,
                    )
                    nc.tensor.matmul(
                        pb[:, cols], lhsT=lt[:, s, 128:256], rhs=et[:, cols],
                        start=True, stop=False,
                    )
                    nc.tensor.matmul(
                        pb[:, cols], lhsT=l2[:, 128:256], rhs=r2[:, s, cols],
                        start=False, stop=True,
                    )
                # ACT evacuations as soon as params land
                mg = work.tile([128, L], BF16, tag="mg", name=f"mg{s}")
                nc.scalar.activation(out=mg, in_=pm, func=AF.Copy)
                mb = work.tile([128, L], BF16, tag="mb", name=f"mb{s}")
                nc.scalar.activation(out=mb, in_=pb, func=AF.Copy)
                pms[s], pbs[s] = mg, mb

            # ---- combines: ot = xm * (1+gamma) + beta; pair 3 goes
            # DVE-direct from PSUM (its ACT evac would queue last), and
            # runs before pair 2 whose evacuations finish last on ACT ----
            for s in range(NPAIR):
                g, i = divmod(s, 2)
                otv = ott[:, g, i * L : (i + 1) * L]
                nc.vector.tensor_tensor(out=otv, in0=xms[s], in1=pms[s], op=OP.mult)
                nc.vector.tensor_tensor(out=otv, in0=otv, in1=pbs[s], op=OP.add)
                nc.sync.dma_start(out=od[g, :, i, :], in_=otv)

    return nc


def _prep_core_inputs(core, x, u_i, e_qid, t, W, V, bias):
    bg, cg = divmod(core, CG)
    bs = slice(bg * BPC, (bg + 1) * BPC)
    rg = slice(cg * CPC, (cg + 1) * CPC)
    rb = slice(C + cg * CPC, C + (cg + 1) * CPC)
    bf = ml_dtypes.bfloat16

    # x / e pair tiles -> groups of 2 pairs side by side
    xp = x[bs, rg, :].reshape(NPAIR, 128, L)
    ep = e_qid[bs].reshape(NPAIR, 128, L)
    xg = np.concatenate([xp[0::2], xp[1::2]], axis=2)   # [2, 128, 2L]
    eg = np.concatenate([ep[0::2], ep[1::2]], axis=2)

    w2 = np.concatenate([W[rg], W[rb]], axis=0)          # (128, 4096) c2=[g|b]
    wr = w2.reshape(128, U, Q)                           # [c2, u, q]
    wt = np.ascontiguousarray(wr.transpose(1, 2, 0)).reshape(64, Q * 128)

    ui_s = np.ascontiguousarray(u_i[bs].T)               # (64, 8)

    vg, vb = V[rg, 0], V[rb, 0]
    bgm, bbt = bias[rg], bias[rb]
    l2 = np.zeros((3, 256), np.float32)
    l2[0, 0:64] = vg
    l2[1, 64:128] = vg
    l2[2, 0:64] = 1.0 + bgm
    l2[2, 64:128] = 1.0 + bgm
    l2[0, 128:192] = vb
    l2[1, 192:256] = vb
    l2[2, 128:192] = bbt
    l2[2, 192:256] = bbt

    r2 = np.empty((3, NPAIR, L), np.float32)
    for s in range(NPAIR):
        r2[0, s] = t[bg * BPC + 2 * s, 0]
        r2[1, s] = t[bg * BPC + 2 * s + 1, 0]
    r2[2] = 1.0
    sm = np.concatenate([r2.reshape(3, NPAIR * L), l2], axis=1)

    return {
        "xg_s": np.ascontiguousarray(xg).astype(bf),
        "eg_s": np.ascontiguousarray(eg).astype(bf),
        "wt_s": wt.astype(bf),
        "ui_s": ui_s.astype(bf),
        "sm2": np.ascontiguousarray(sm, dtype=np.float32),
    }


def kernel(x, u_i, e_qid, t, W, V, bias):
    x = np.asarray(x, np.float32)
    u_i = np.asarray(u_i, np.float32)
    e_qid = np.asarray(e_qid, np.float32)
    t = np.asarray(t, np.float32)
    W = np.asarray(W, np.float32)
    V = np.asarray(V, np.float32)
    bias = np.asarray(bias, np.float32)

    if "nc" not in _CACHE:
        _CACHE["nc"] = _build_nc()
    nc = _CACHE["nc"]

    in_maps = [
        _prep_core_inputs(i, x, u_i, e_qid, t, W, V, bias) for i in range(N_CORES)
    ]
    results = run_bass_kernel_spmd(nc, in_maps, list(range(N_CORES))).results

    out = np.empty((B, C, L), np.float32)
    for i in range(N_CORES):
        bg, cg = divmod(i, CG)
        res = np.asarray(results[i]["out_s"], dtype=np.float32)  # [2, 128, 2L]
        # [g, (sp c), (i l)] -> sample b = 4g + 2i + sp
        res = res.reshape(2, 2, CPC, 2, L).transpose(0, 3, 1, 2, 4)
        out[bg * BPC : (bg + 1) * BPC, cg * CPC : (cg + 1) * CPC, :] = res.reshape(
            BPC, CPC, L
        )
    return out


# revision 14
# speedup vs baseline: 1.0719x; 1.0353x over previous
"""ConditionAwareAdaIN Trainium2 kernel (bf16 rewrite).

Reference computation (B=16, C=256, L=1024, U=64, Q=64):
    nx    = InstanceNorm1d(x)                       # per-(b,c) stats over L
    A     = einsum('bu,cuq->bcq', u_i, W.reshape(2C,U,Q))
    style = einsum('bcq,bql->bcl', A, e_qid)
    gamma, beta = split(style + V@t + bias, 2, axis=1)
    out   = (1 + gamma) * nx + beta

Sharding: 2-way over batch x 4-way over channels -> 8 cores, each owning
8 samples x 64 channels (its slice of gamma AND beta rows of W/V/bias).

All bulk tensors move as bf16 (DMA time on the cost model is serialized
bytes/360GB/s, so halving bytes halves the DMA floor); PSUM accumulation
stays fp32, output is bf16 and upcast on host.

Per-core device kernel:
  stage 1: A in c2-major layout via 64 tiny matmuls (one per q):
           pa[c2, (s, sp, q)] += wt_q[u, c2]^T @ ui[u, b].  PE transposes
           (one per sample pair) then produce the block-diagonal stage-2
           lhsT with q on partitions; no DRAM bounce, tiny evacuations.
  stage 2: per sample-pair, block-diagonal style matmuls (K=128) + a K=3
           accumulating matmul folding V*t, bias and the "+1" of
           (1+gamma).
  norm:    bn_stats/bn_aggr per pair tile (2 samples x 64 ch = 128 rows);
           xm = (x - mean) * rstd on DVE (4x bf16 mode), params
           evacuated PSUM->bf16 on ACT, then out = xm*(1+gamma) + beta
           as two 2x-mode DVE tensor_tensor ops.
"""

import json

import numpy as np
import ml_dtypes

for _p in ("/opt/trn_rl_repo", "/root/.axon_site/_ro/trn_rl_repo"):
    import sys as _sys

    if _p not in _sys.path:
        _sys.path.append(_p)

import concourse.bass as bass
import concourse.mybir as mybir
from concourse.tile import TileContext
from concourse.bass_utils import run_bass_kernel_spmd
from concourse import masks


def _split_sync_waits(raw: bytes, keep: int = 1) -> bytes:
    """Walrus in this env accepts at most one sync wait per TPB instruction.

    Tile packs several waits into sync_info.on_wait; re-emit the excess as
    standalone single-wait EventSemaphore instructions (what wait_ge emits)
    immediately before the instruction, in the same engine stream.
    """
    bir = json.loads(raw)
    n = 0
    for fn in bir["functions"]:
        for blk in fn["blocks"]:
            out = []
            for ins in blk["instructions"]:
                si = ins.get("sync_info")
                ws = si.get("on_wait") if si else None
                if ws and len(ws) > keep:
                    for w in ws[: len(ws) - keep]:
                        n += 1
                        out.append(
                            {
                                "debug": ins.get("debug", 0),
                                "engine": ins["engine"],
                                "ins": [],
                                "outs": [],
                                "name": f"evw-{n}",
                                "opcode": "EventSemaphore",
                                "sync_info": {"on_update": [], "on_wait": [w]},
                            }
                        )
                    si["on_wait"] = ws[len(ws) - keep :]
                out.append(ins)
            blk["instructions"] = out
    return json.dumps(bir).encode()


class _Bass(bass.Bass):
    def to_json_bytes(self) -> bytes:
        return _split_sync_waits(super().to_json_bytes())


B, C, L = 16, 256, 1024
U, Q = 64, 64
EPS = 1e-5
N_CORES = 8
BG, CG = 2, 4          # batch groups x channel groups
BPC = B // BG          # samples per core = 8
CPC = C // CG          # channels per core = 64
NPAIR = BPC // 2       # sample pairs per core = 4

FP32 = mybir.dt.float32
BF16 = mybir.dt.bfloat16

_CACHE = {}


def _build_nc(detect_races: bool = True):
    nc = _Bass(detect_race_conditions=detect_races)

    # x / e pair-group tiles: [g, 128 rows (2 samples x 64 ch/q), 2 pairs x L]
    xg_in = nc.dram_tensor("xg_s", [2, 128, 2 * L], BF16, kind="ExternalInput")
    eg_in = nc.dram_tensor("eg_s", [2, 128, 2 * L], BF16, kind="ExternalInput")
    # wt: stage-1 lhsT per q: wt[u, q*128 + c2] = W2[c2, u, q]
    wt_in = nc.dram_tensor("wt_s", [64, 64 * 128], BF16, kind="ExternalInput")
    ui_in = nc.dram_tensor("ui_s", [64, BPC], BF16, kind="ExternalInput")
    # sm: [r2 (4 pairs x 1024) | l2 (256)] on 3 partitions (fp32)
    sm_in = nc.dram_tensor("sm2", [3, NPAIR * L + 256], FP32, kind="ExternalInput")
    out_d = nc.dram_tensor("out_s", [2, 128, 2 * L], BF16, kind="ExternalOutput")

    AF = mybir.ActivationFunctionType
    OP = mybir.AluOpType
    F32R = mybir.dt.float32r

    with TileContext(nc) as tc:
        with (
            tc.tile_pool(name="persist", bufs=1) as persist,
            tc.tile_pool(name="stat", bufs=8) as stat,
            tc.tile_pool(name="work", bufs=4) as work,
            tc.tile_pool(name="psM", bufs=4, space="PSUM") as psM,
        ):
            # small local tiles first (no DMA deps) so PE warm-up can start
            dum = persist.tile([64, 512], BF16, tag="dum")
            nc.vector.memset(dum, 0.0)
            eps_t = persist.tile([128, 1], FP32, tag="eps")
            nc.vector.memset(eps_t, EPS)
            idn = persist.tile([128, 128], BF16, tag="idn")
            masks.make_identity(nc, idn[:, :])
            # block-diagonal stage-2 lhsT, zero blocks stay zero
            lt = persist.tile([128, NPAIR, 256], BF16, tag="lt")
            nc.gpsimd.memset(lt[:, :, :], 0.0)

            # ---- input DMAs (DMA device is serialized; order matters):
            # ui (tiny) -> x group 0 (unblocks stats) -> wt (unblocks the
            # stage-1 -> lt chain) -> e group 0 + sm (unblocks stage 2) ->
            # x/e group 1.
            sm = persist.tile([3, NPAIR * L + 256], F32R, tag="sm")
            nc.sync.dma_start(out=sm, in_=sm_in[:, :].bitcast(F32R))
            r2 = sm[:, 0 : NPAIR * L].rearrange("k (s l) -> k s l", s=NPAIR)
            l2 = sm[:, NPAIR * L : NPAIR * L + 256]
            uit = persist.tile([64, BPC], BF16, tag="uit")
            nc.sync.dma_start(out=uit, in_=ui_in[:, :])
            xgt = persist.tile([128, 2, 2 * L], BF16, tag="xgt")
            egt = persist.tile([128, 2, 2 * L], BF16, tag="egt")
            nc.sync.dma_start(out=xgt[:, 0, :], in_=xg_in[0, :, :])
            wt = persist.tile([64, 64 * 128], BF16, tag="wt")
            nc.sync.dma_start(out=wt[:, 0 : 32 * 128], in_=wt_in[:, 0 : 32 * 128])
            nc.sync.dma_start(out=wt[:, 32 * 128 :], in_=wt_in[:, 32 * 128 :])
            nc.sync.dma_start(out=egt[:, 0, :], in_=eg_in[0, :, :])
            nc.sync.dma_start(out=egt[:, 1, :], in_=eg_in[1, :, :])
            nc.sync.dma_start(out=xgt[:, 1, 0:L], in_=xg_in[1, :, 0:L])
            nc.sync.dma_start(out=xgt[:, 1, L:], in_=xg_in[1, :, L:])

            ott = persist.tile([128, 2, 2 * L], BF16, tag="ott")

            # ---- PE warm-up: start the p-state ramp clock early ----
            # pa cells are all rewritten with start=True by stage 1.
            pa = psM.tile([128, 4, 2, 64], FP32, tag="ps", name="pa")
            paw = pa.rearrange("p s h q -> p (s h q)")
            for wu in range(3):
                nc.tensor.matmul(
                    paw[0:8, 0:512],
                    lhsT=dum[:, 0:8],
                    rhs=dum[:, 0:512],
                    start=True,
                    stop=True,
                )

            # ---- stage 1: pa[c2, (s, sp, q)] = sum_u wt_q[u, c2] ui[u, b] ----
            # b = 2s + sp; out free dims (s:4, sp:2) are strided, offset q.
            a_sb = persist.tile([128, 512], BF16, tag="a_sb")
            asv = a_sb.rearrange("p (s h q) -> p s h q", s=NPAIR, h=2)
            for qh in range(2):
                for q in range(qh * 32, qh * 32 + 32):
                    nc.tensor.matmul(
                        pa[:, :, :, q],
                        lhsT=wt[:, q * 128 : (q + 1) * 128],
                        rhs=uit.rearrange("u (s h) -> u s h", s=NPAIR),
                        start=True,
                        stop=True,
                    )
                # evacuate A half to SBUF bf16, layout [c2, (s, sp*64+q)]
                nc.scalar.activation(
                    out=asv[:, :, :, qh * 32 : qh * 32 + 32],
                    in_=pa[:, :, :, qh * 32 : qh * 32 + 32],
                    func=AF.Copy,
                )

            # keep the PE p-state ramp alive across the stage-1 gap
            pw = psM.tile([8, 512], FP32, tag="ps", name="pw")
            for wu in range(3):
                nc.tensor.matmul(
                    pw, lhsT=dum[:, 0:8], rhs=dum[:, 0:512], start=True, stop=True
                )

            # per pair: PE transpose [c2, (sp,q)] -> [(sp,q), c2] (bf16 PSUM)
            # same tag as pa: reuses its bank once evac'd
            pt = psM.tile([128, 4, 128], BF16, tag="ps", name="pt")
            for s in range(NPAIR):
                nc.tensor.transpose(
                    pt[:, s, :], a_sb[:, s * 128 : (s + 1) * 128], idn[:, :]
                )
            # scatter into the block-diagonal lhsT:
            #   lt[sp*64+q, s, gb*128 + sp*64 + c'] = A[2s+sp, gb*64+c', q]
            lt4 = lt.rearrange("p s (gb c) -> p s gb c", gb=2)
            pt4 = pt.rearrange("p s (gb c) -> p s gb c", gb=2)
            # more ramp fillers while lt is being scattered on DVE
            for wu in range(3):
                nc.tensor.matmul(
                    pw, lhsT=dum[:, 0:8], rhs=dum[:, 0:512], start=True, stop=True
                )

            # (scatter into lt happens on DVE, emitted inside the stats
            # loop below so it lands in the DVE queue's idle gap between
            # the group-0 and DMA-gated group-1 stats)

            # ---- norm stats for all pairs first (keeps the DVE queue
            # free of combine ops that wait on ACT evacuations) ----
            od = out_d.rearrange("g p (i l) -> g p i l", i=2)
            xms, mvs, rstds = [], [], []
            for s in range(NPAIR):
                g, i = divmod(s, 2)
                xt = xgt[:, g, i * L : (i + 1) * L]
                st = stat.tile([128, 2, 6], FP32, tag="st")
                nc.vector.bn_stats(st[:, 0, :], xt[:, 0:512])
                nc.vector.bn_stats(st[:, 1, :], xt[:, 512:1024])
                mv = stat.tile([128, 2], FP32, tag="mv")
                nc.vector.bn_aggr(mv, st)
                rstd = stat.tile([128, 1], FP32, tag="rstd")
                nc.scalar.activation(
                    out=rstd, in_=mv[:, 1:2], func=AF.Sqrt, bias=eps_t, scale=1.0
                )
                nc.vector.reciprocal(rstd, rstd)
                xm = work.tile([128, L], BF16, tag="xm", name=f"xm{s}")
                nc.vector.tensor_scalar(
                    out=xm,
                    in0=xt,
                    scalar1=mv[:, 0:1],
                    scalar2=rstd,
                    op0=OP.subtract,
                    op1=OP.mult,
                )
                xms.append(xm)
                if s == 1:
                    for sp in range(2):
                        rows = slice(sp * 64, sp * 64 + 64)
                        nc.vector.tensor_copy(
                            out=lt4[rows, :, :, sp * 64 : sp * 64 + 64],
                            in_=pt4[rows, :, :, :],
                        )

            # ---- stage 2 matmuls per pair ----
            pms, pbs = [], []
            for s in range(NPAIR):
                g, i = divmod(s, 2)
                et = egt[:, g, i * L : (i + 1) * L]
                pm = psM.tile([128, L], FP32, tag="ps", name=f"pm{s}")
                pb = psM.tile([128, L], FP32, tag="ps", name=f"pb{s}")
                pms.append(pm)
                pbs.append(pb)
                for h in range(2):
                    cols = slice(h * 512, (h + 1) * 512)
                    nc.tensor.matmul(
                        pm[:, cols], lhsT=lt[:, s, 0:128], rhs=et[:, cols],
                        start=True, stop=False,
                    )
                    nc.tensor.matmul(
                        pm[:, cols], lhsT=l2[:, 0:128], rhs=r2[:, s, cols],
                        start=False, stop=True,
                    )
                    nc.tensor.matmul(
                        pb[:, cols], lhsT=lt[:, s, 128:256], rhs=et[:, cols],
                        start=True, stop=False,
                    )
                    nc.tensor.matmul(
                        pb[:, cols], lhsT=l2[:, 128:256], rhs=r2[:, s, cols],
                        start=False, stop=True,
                    )
                # ACT evacuations as soon as params land
                mg = work.tile([128, L], BF16, tag="mg", name=f"mg{s}")
                nc.scalar.activation(out=mg, in_=pm, func=AF.Copy)
                mb = work.tile([128, L], BF16, tag="mb", name=f"mb{s}")
                nc.scalar.activation(out=mb, in_=pb, func=AF.Copy)
                pms[s], pbs[s] = mg, mb

            # ---- combines: ot = xm * (1+gamma) + beta; pair 3 goes
            # DVE-direct from PSUM (its ACT evac would queue last), and
            # runs before pair 2 whose evacuations finish last on ACT ----
            for s in range(NPAIR):
                g, i = divmod(s, 2)
                otv = ott[:, g, i * L : (i + 1) * L]
                nc.vector.tensor_tensor(out=otv, in0=xms[s], in1=pms[s], op=OP.mult)
                nc.vector.tensor_tensor(out=otv, in0=otv, in1=pbs[s], op=OP.add)
                nc.sync.dma_start(out=od[g, :, i, :], in_=otv)

    return nc


def _prep_core_inputs(core, x, u_i, e_qid, t, W, V, bias):
    bg, cg = divmod(core, CG)
    bs = slice(bg * BPC, (bg + 1) * BPC)
    rg = slice(cg * CPC, (cg + 1) * CPC)
    rb = slice(C + cg * CPC, C + (cg + 1) * CPC)
    bf = ml_dtypes.bfloat16

    # x / e pair tiles -> groups of 2 pairs side by side
    xp = x[bs, rg, :].reshape(NPAIR, 128, L)
    ep = e_qid[bs].reshape(NPAIR, 128, L)
    xg = np.concatenate([xp[0::2], xp[1::2]], axis=2)   # [2, 128, 2L]
    eg = np.concatenate([ep[0::2], ep[1::2]], axis=2)

    w2 = np.concatenate([W[rg], W[rb]], axis=0)          # (128, 4096) c2=[g|b]
    wr = w2.reshape(128, U, Q)                           # [c2, u, q]
    wt = np.ascontiguousarray(wr.transpose(1, 2, 0)).reshape(64, Q * 128)

    ui_s = np.ascontiguousarray(u_i[bs].T)               # (64, 8)

    vg, vb = V[rg, 0], V[rb, 0]
    bgm, bbt = bias[rg], bias[rb]
    l2 = np.zeros((3, 256), np.float32)
    l2[0, 0:64] = vg
    l2[1, 64:128] = vg
    l2[2, 0:64] = 1.0 + bgm
    l2[2, 64:128] = 1.0 + bgm
    l2[0, 128:192] = vb
    l2[1, 192:256] = vb
    l2[2, 128:192] = bbt
    l2[2, 192:256] = bbt

    r2 = np.empty((3, NPAIR, L), np.float32)
    for s in range(NPAIR):
        r2[0, s] = t[bg * BPC + 2 * s, 0]
        r2[1, s] = t[bg * BPC + 2 * s + 1, 0]
    r2[2] = 1.0
    sm = np.concatenate([r2.reshape(3, NPAIR * L), l2], axis=1)

    return {
        "xg_s": np.ascontiguousarray(xg).astype(bf),
        "eg_s": np.ascontiguousarray(eg).astype(bf),
        "wt_s": wt.astype(bf),
        "ui_s": ui_s.astype(bf),
        "sm2": np.ascontiguousarray(sm, dtype=np.float32),
    }


def kernel(x, u_i, e_qid, t, W, V, bias):
    x = np.asarray(x, np.float32)
    u_i = np.asarray(u_i, np.float32)
    e_qid = np.asarray(e_qid, np.float32)
    t = np.asarray(t, np.float32)
    W = np.asarray(W, np.float32)
    V = np.asarray(V, np.float32)
    bias = np.asarray(bias, np.float32)

    if "nc" not in _CACHE:
        _CACHE["nc"] = _build_nc()
    nc = _CACHE["nc"]

    in_maps = [
        _prep_core_inputs(i, x, u_i, e_qid, t, W, V, bias) for i in range(N_CORES)
    ]
    results = run_bass_kernel_spmd(nc, in_maps, list(range(N_CORES))).results

    out = np.empty((B, C, L), np.float32)
    for i in range(N_CORES):
        bg, cg = divmod(i, CG)
        res = np.asarray(results[i]["out_s"], dtype=np.float32)  # [2, 128, 2L]
        # [g, (sp c), (i l)] -> sample b = 4g + 2i + sp
        res = res.reshape(2, 2, CPC, 2, L).transpose(0, 3, 1, 2, 4)
        out[bg * BPC : (bg + 1) * BPC, cg * CPC : (cg + 1) * CPC, :] = res.reshape(
            BPC, CPC, L
        )
    return out
